# revision 28
# baseline (speedup 1.0000x reference)
"""Differential attention TRN2 Bass kernel.

Math (per batch b, 8 heads, hd=48, hd2=24, N=1024):
  qkv = qkv_w @ x ; q1,q2,k1,k2,v1,v2 head-split
  q2n = GroupNorm2(q2)*gn_w + gn_b
  attn_i = softmax over keys; diff = attn1 - attn2; out = cat(diff@v1, diff@v2)
  y = proj_w @ scramble(out) + proj_b

Sharding: pure data parallel, one batch element per NeuronCore (B=8 = n_cores).

Device dataflow per core (validated bit-for-bit in numpy against the jax
reference before porting):
  * qk-projection matmul emits a padded per-head layout (32-row blocks, 24
    used) so per-head score matmuls can be row-tiled 4-at-a-time (K=32).
  * scores are computed transposed, scoresT[m, n] = k^T q, so softmax
    normalization can be deferred: w_i = [v1;v2;ones]^T @ exp(scoresT_i)
    yields both the weighted values and the softmax denominators (ones row)
    in one accumulated matmul chain over key chunks.
  * exp() runs on ACT from PSUM in [128, 1024] strips (head pairs, both
    softmax paths interleaved per key chunk, double-buffered score PSUM so
    the ACT exp stream - the bottleneck engine - never stalls).
  * a small [128, 512] PE transpose of w puts n on partitions; DVE applies
    1/D rescale + path subtraction via free-dim-broadcast tensor_tensor.
  * torch's cat(dim=2)+reshape channel scramble is realized for free by
    DMA-ing out_nd row-major into a DRAM scratch and re-reading it as
    [channels, N] for the projection matmul.
  * scale and scale*lambda_h are folded into the qk weights / GN affine on
    the host; softmax max-subtraction is skipped (score std ~0.11, safe).
  * big matmuls run as float32r (full PE rate; fp32 is 4 cycles/row).
"""
import sys

if '/opt/trn_rl_repo' not in sys.path:
    sys.path.insert(0, '/opt/trn_rl_repo')

import numpy as np

import concourse.bass as bass
import concourse.bacc as bacc
import concourse.mybir as mybir
import concourse.tile as tile
from concourse import bass_utils, masks

NH, HD, HD2, C, N = 8, 48, 24, 384, 1024
SCALE = HD ** -0.5
EPS = 1e-5
F32 = mybir.dt.float32
MMD = mybir.dt.float32r
AF = mybir.ActivationFunctionType
ALU = mybir.AluOpType


def build_nc(reps=1, loop_reps=0, ablate=()):
    nc = bacc.Bacc("TRN2", target_bir_lowering=False, debug=False)

    x_d = nc.dram_tensor("x", [C, N], MMD, kind="ExternalInput")
    wqk_d = nc.dram_tensor("wqk", [C, 1024], MMD, kind="ExternalInput")
    wv_d = nc.dram_tensor("wv", [C, 512], MMD, kind="ExternalInput")
    wproj_d = nc.dram_tensor("wproj", [C, C], MMD, kind="ExternalInput")
    pbias_d = nc.dram_tensor("pbias", [C, 1], F32, kind="ExternalInput")
    ga_d = nc.dram_tensor("ga", [2, 128, 1], F32, kind="ExternalInput")
    gb_d = nc.dram_tensor("gb", [2, 128, 1], F32, kind="ExternalInput")
    out_d = nc.dram_tensor("out", [C, N], F32, kind="ExternalOutput")
    scram_d = nc.dram_tensor("scram", [C * N], MMD)

    with tile.TileContext(nc) as tc:
        with (
            tc.tile_pool(name="persist", bufs=1) as pp,
            tc.tile_pool(name="work", bufs=1) as wp,
            tc.tile_pool(name="epool", bufs=8) as ep,
            tc.tile_pool(name="small", bufs=4) as sp,
            tc.tile_pool(name="psS", bufs=2, space=bass.MemorySpace.PSUM) as psS,
            tc.tile_pool(name="psW", bufs=4, space=bass.MemorySpace.PSUM) as psW,
        ):
            # ---- persistent loads (spread across engine DMA queues) ----
            def load(name, dram, shape, n_tiles, cols, eng, dt=MMD):
                ts = []
                for i in range(n_tiles):
                    t = pp.tile(shape, dt, name=f"{name}{i}", tag=f"{name}{i}")
                    eng.dma_start(t[:], dram[128 * i:128 * (i + 1), 0:cols])
                    ts.append(t)
                return ts

            xs, wqk = [], []
            engs = [nc.sync, nc.gpsimd, nc.sync]
            for i in range(3):
                t = pp.tile([128, N], MMD, name=f"x{i}", tag=f"x{i}")
                engs[i].dma_start(t[:], x_d[128 * i:128 * (i + 1), :])
                xs.append(t)
                t = pp.tile([128, 1024], MMD, name=f"wqk{i}", tag=f"wqk{i}")
                engs[2 - i].dma_start(t[:], wqk_d[128 * i:128 * (i + 1), :])
                wqk.append(t)
            wv = load("wv", wv_d, [128, 512], 3, 512, nc.sync)
            wproj = load("wproj", wproj_d, [128, C], 3, C, nc.gpsimd)
            pb = []
            for i in range(3):
                t = pp.tile([128, 1], F32, name=f"pb{i}", tag=f"pb{i}")
                nc.gpsimd.dma_start(t[:], pbias_d[128 * i:128 * (i + 1), :])
                pb.append(t)
            ga, gb = [], []
            for g in range(2):
                ta = pp.tile([128, 1], F32, name=f"ga{g}", tag=f"ga{g}")
                nc.gpsimd.dma_start(ta[:], ga_d[g])
                tb = pp.tile([128, 1], F32, name=f"gb{g}", tag=f"gb{g}")
                nc.gpsimd.dma_start(tb[:], gb_d[g])
                ga.append(ta)
                gb.append(tb)

            zb = pp.tile([128, 1], F32, name="zb", tag="zb")
            nc.gpsimd.memset(zb[:], 0.0)
            ones_col = pp.tile([128, 1], F32, name="ones_col", tag="ones_col")
            nc.gpsimd.memset(ones_col[:], 1.0)
            ones_row = pp.tile([1, 128], F32, name="ones_row", tag="ones_row")
            nc.gpsimd.memset(ones_row[:], 1.0)
            ident = pp.tile([128, 128], F32, name="ident", tag="ident")
            masks.make_identity(nc, ident[:])
            eps_t = pp.tile([1, 1], F32, name="eps_t", tag="eps_t")
            nc.gpsimd.memset(eps_t[:], EPS)

            def emit_body(sfx):
                # ---- qk projection: 8 M-tiles x 2 free halves ----
                # tiles: 0=q1a 1=q1b 2=q2a 3=q2b 4=k1a 5=k1b 6=k2a 7=k2b
                names = ["q1a", "q1b", "q2a", "q2b", "k1a", "k1b", "k2a", "k2b"]
                order = [2, 0, 4, 3, 6, 1, 5, 7]  # GN + first-pair critical first
                qk = {}
                for t in range(8):
                    qk[names[t]] = wp.tile([128, 1024], MMD,
                                           name=names[t] + sfx, tag=names[t])
                slices = [(t, f) for t in order for f in range(2)]
                for grp in range(8):
                    ps = psS.tile([128, 1024], F32, name=f"qkps{grp}{sfx}",
                                  tag="ps_s")
                    for j in range(2):
                        t, f = slices[2 * grp + j]
                        for k in range(3):
                            nc.tensor.matmul(
                                ps[:, 512 * j:512 * (j + 1)],
                                (wqk[k][:, 128 * t:128 * (t + 1)]),
                                (xs[k][:, 512 * f:512 * (f + 1)]),
                                start=(k == 0), stop=(k == 2))
                    for j in range(2):
                        t, f = slices[2 * grp + j]
                        nc.vector.tensor_copy(
                            qk[names[t]][:, 512 * f:512 * (f + 1)],
                            ps[:, 512 * j:512 * (j + 1)])

                # ---- vT projection: vT[m, 64h+d] ----
                vt = []
                for mt in range(8):
                    vt.append(wp.tile([128, 512], MMD, name=f"vt{mt}{sfx}",
                                      tag=f"vt{mt}"))
                for grp in range(4):
                    ps = psS.tile([128, 1024], F32, name=f"vtps{grp}{sfx}",
                                  tag="ps_s")
                    for j in range(2):
                        mt = 2 * grp + j
                        for k in range(3):
                            nc.tensor.matmul(
                                ps[:, 512 * j:512 * (j + 1)],
                                (xs[k][:, 128 * mt:128 * (mt + 1)]),
                                (wv[k][:]),
                                start=(k == 0), stop=(k == 2))
                    for j in range(2):
                        mt = 2 * grp + j
                        nc.vector.tensor_copy(vt[mt][:], ps[:, 512 * j:512 * (j + 1)])
                for mt in range(8):
                    for h in range(NH):
                        nc.gpsimd.tensor_copy(
                            vt[mt][:, 64 * h + 48:64 * h + 49], ones_col[:])

                # ---- GroupNorm stats + affine ----
                q2n = []
                NG = 96.0 * N
                for g in range(2):
                    q2r = qk["q2a" if g == 0 else "q2b"]
                    st = sp.tile([128, 2], F32, name=f"st{g}{sfx}", tag="st", bufs=2)
                    nc.vector.reduce_sum(st[:, 0:1], q2r[:], axis=mybir.AxisListType.X)
                    sqs = wp.tile([128, 1024], F32, name=f"sqs{g}{sfx}", tag="sqs")
                    nc.vector.scalar_tensor_tensor(
                        sqs[:], q2r[:], 1.0, q2r[:],
                        op0=ALU.mult, op1=ALU.mult, accum_out=st[:, 1:2])
                    red = psW.tile([128, 512], F32, name=f"red{g}{sfx}", tag="w")
                    nc.tensor.matmul(red[0:1, 0:2], ones_col[:], st[:],
                                     start=True, stop=True)
                    bcv = sp.tile([1, 2], F32, name=f"bcv{g}{sfx}", tag="bcv", bufs=2)
                    # bcv = [-mean, rstd]
                    nc.scalar.mul(bcv[0:1, 0:1], red[0:1, 0:1], -1.0 / NG)
                    q2m = sp.tile([1, 1], F32, name=f"q2m{g}{sfx}", tag="q2m", bufs=2)
                    nc.scalar.mul(q2m[:], red[0:1, 1:2], 1.0 / NG)
                    m2 = sp.tile([1, 1], F32, name=f"m2{g}{sfx}", tag="m2", bufs=2)
                    nc.vector.tensor_tensor(m2[:], bcv[0:1, 0:1], bcv[0:1, 0:1],
                                            op=ALU.mult)
                    var = sp.tile([1, 1], F32, name=f"var{g}{sfx}", tag="var", bufs=2)
                    nc.vector.tensor_tensor(var[:], q2m[:], m2[:], op=ALU.subtract)
                    # rstd = exp(-0.5*ln(var+eps)): stays in the ln/exp ACT
                    # table set (Sqrt would force two table reloads per pass)
                    lnv = sp.tile([1, 1], F32, name=f"lnv{g}{sfx}", tag="lnv", bufs=2)
                    nc.scalar.activation(lnv[:], var[:], AF.Ln, bias=eps_t[:])
                    nc.scalar.activation(bcv[0:1, 1:2], lnv[:], AF.Exp, scale=-0.5)
                    bc = psW.tile([128, 512], F32, name=f"bc{g}{sfx}", tag="w")
                    nc.tensor.matmul(bc[:, 0:2], ones_row[:], bcv[:],
                                     start=True, stop=True)
                    A = sp.tile([128, 1], F32, name=f"A{g}{sfx}", tag="A", bufs=2)
                    nc.vector.tensor_tensor(A[:], ga[g][:], bc[:, 1:2], op=ALU.mult)
                    B = sp.tile([128, 1], F32, name=f"B{g}{sfx}", tag="B", bufs=2)
                    nc.vector.scalar_tensor_tensor(
                        B[:], A[:], bc[:, 0:1], gb[g][:],
                        op0=ALU.mult, op1=ALU.add)
                    qn = wp.tile([128, 1024], MMD, name=f"q2n{g}{sfx}", tag=f"q2n{g}")
                    nc.vector.tensor_scalar(qn[:], q2r[:], A[:], B[:],
                                            op0=ALU.mult, op1=ALU.add)
                    q2n.append(qn)

                # ---- main attention loop ----
                # head pairs; both softmax paths interleaved per key-chunk so
                # ACT's exp stream never waits (double-buffered score PSUM).
                out_nd = []
                for h in range(NH):
                    out_nd.append(wp.tile([128, 8, 48], MMD,
                                          name=f"ond{h}{sfx}", tag=f"ond{h}"))
                scx = []

                def emit_scx(cc):
                    t = wp.tile([128, 1024], MMD, name=f"scx{cc}{sfx}",
                                tag=f"scx{cc}")
                    src = scram_d[131072 * cc:131072 * (cc + 1)].rearrange(
                        "(p n) -> p n", n=N)
                    nc.sync.dma_start(t[:], src)
                    scx.append(t)

                for hg in range(2):
                    k1 = qk["k1a" if hg == 0 else "k1b"]
                    k2 = qk["k2a" if hg == 0 else "k2b"]
                    q1 = qk["q1a" if hg == 0 else "q1b"]
                    q2 = q2n[hg]
                    for pr in range(2):  # head pair within group
                        heads = (4 * hg + 2 * pr, 4 * hg + 2 * pr + 1)
                        prow = 64 * pr  # row base of this pair in qk tiles
                        for nt in range(2):
                            ncols = slice(512 * nt, 512 * (nt + 1))
                            wt = {}
                            for j in range(2):
                                for path in range(2):
                                    wt[j, path] = psW.tile(
                                        [64, 512], F32,
                                        name=f"w{hg}{pr}{nt}{j}{path}{sfx}",
                                        tag="w")
                            for mc in range(8):
                                mcols = slice(128 * mc, 128 * (mc + 1))
                                E = {}
                                for path, (kk, qq) in enumerate(
                                        [(k1, q1), (k2, q2)]):
                                    ps = psS.tile(
                                        [128, 1024], F32,
                                        name=f"s{hg}{pr}{nt}{mc}{path}{sfx}",
                                        tag="ps_s")
                                    for j in range(2):
                                        rows = slice(prow + 32 * j,
                                                     prow + 32 * j + 32)
                                        nc.tensor.matmul(
                                            ps[:, 512 * j:512 * (j + 1)],
                                            kk[rows, mcols], qq[rows, ncols],
                                            start=True, stop=True,
                                            tile_position=(prow + 32 * j, 0))
                                    Et = ep.tile(
                                        [128, 1024], MMD,
                                        name=f"E{hg}{pr}{nt}{mc}{path}{sfx}",
                                        tag="E")
                                    if 'noexp' in ablate:
                                        nc.scalar.activation(
                                            Et[0:1, 0:1], ps[0:1, 0:1],
                                            AF.Exp, bias=zb[0:1, :])
                                    else:
                                        nc.scalar.activation(Et[:], ps[:],
                                                             AF.Exp, bias=zb[:])
                                    E[path] = Et
                                for j in range(2):
                                    h = heads[j]
                                    for path in range(2):
                                        nc.tensor.matmul(
                                            wt[j, path][:],
                                            vt[mc][:, 64 * h:64 * h + 64],
                                            E[path][:, 512 * j:512 * (j + 1)],
                                            start=(mc == 0), stop=(mc == 7),
                                            skip_group_check=True)
                            # tail: stack w paths, transpose, rescale, subtract
                            for j in range(2):
                                h = heads[j]
                                stg = wp.tile([128, 512], F32,
                                              name=f"stg{hg}{pr}{nt}{j}{sfx}",
                                              tag=f"stg{j}", bufs=2)
                                for path in range(2):
                                    nc.vector.tensor_copy(
                                        stg[64 * path:64 * path + 64, :],
                                        wt[j, path][:])
                                tr = psW.tile([128, 512], F32,
                                              name=f"tr{hg}{pr}{nt}{j}{sfx}",
                                              tag="w")
                                for i in range(4):
                                    cols = slice(128 * i, 128 * (i + 1))
                                    if 'notrans' in ablate:
                                        nc.scalar.copy(tr[:, cols], stg[:, cols])
                                    else:
                                        nc.tensor.transpose(tr[:, cols],
                                                            stg[:, cols], ident[:])
                                rr = sp.tile([128, 2, 4], F32,
                                             name=f"rr{hg}{pr}{nt}{j}{sfx}",
                                             tag="rr", bufs=4)
                                nc.vector.reciprocal(
                                    rr[:, 0, :], tr[:, 48:512:128])
                                nc.vector.reciprocal(
                                    rr[:, 1, :], tr[:, 112:512:128])
                                tr3 = tr.rearrange("p (b c) -> p b c", b=4)
                                rb1 = rr[:, 0:1, :].rearrange(
                                    "p o b -> p b o").broadcast_to([128, 4, 48])
                                rb2 = rr[:, 1:2, :].rearrange(
                                    "p o b -> p b o").broadcast_to([128, 4, 48])
                                tmp = sp.tile([128, 4, 48], F32,
                                              name=f"tm{hg}{pr}{nt}{j}{sfx}",
                                              tag="tmp", bufs=4)
                                nc.vector.tensor_tensor(
                                    tmp[:], tr3[:, :, 64:112], rb2, op=ALU.mult)
                                t2 = sp.tile([128, 4, 48], F32,
                                             name=f"t2{hg}{pr}{nt}{j}{sfx}",
                                             tag="t2", bufs=4)
                                nc.vector.tensor_tensor(
                                    t2[:], tr3[:, :, 0:48], rb1, op=ALU.mult)
                                nc.vector.tensor_tensor(
                                    out_nd[h][:, 4 * nt:4 * nt + 4, :],
                                    t2[:], tmp[:], op=ALU.subtract)
                        # per-pair scramble DMA (both n-tiles done)
                        for j in range(2):
                            h = heads[j]
                            eng = nc.sync if j == 0 else nc.gpsimd
                            for half in range(2):
                                off = h * HD * N + half * HD2 * N
                                dst = scram_d[off:off + HD2 * N].rearrange(
                                    "(t p d) -> p t d", p=128, d=HD2)
                                eng.dma_start(
                                    dst, out_nd[h][:, :, 24 * half:24 * half + 24])
                        if hg == 0 and pr == 1:
                            emit_scx(0)  # heads 0-2 complete
                        elif hg == 1 and pr == 0:
                            emit_scx(1)  # heads 2-5 complete
                        elif hg == 1 and pr == 1:
                            emit_scx(2)
                for ot in range(3):
                    outf = wp.tile([128, 1024], F32, name=f"outf{ot}{sfx}",
                                   tag=f"outf{ot}")
                    for f in range(2):
                        ps = psW.tile([128, 512], F32, name=f"pj{ot}{f}{sfx}",
                                      tag="w")
                        for cc in range(3):
                            nc.tensor.matmul(
                                ps[:],
                                (wproj[cc][:, 128 * ot:128 * (ot + 1)]),
                                (scx[cc][:, 512 * f:512 * (f + 1)]),
                                start=(cc == 0), stop=(cc == 2))
                        nc.vector.tensor_scalar(
                            outf[:, 512 * f:512 * (f + 1)], ps[:],
                            pb[ot][:], None, op0=ALU.add)
                        eng = nc.sync if f == 0 else nc.gpsimd
                        eng.dma_start(
                            out_d[128 * ot:128 * (ot + 1),
                                  512 * f:512 * (f + 1)],
                            outf[:, 512 * f:512 * (f + 1)])

            if loop_reps > 1:
                with tc.For_i(0, loop_reps, 1):
                    emit_body("_L")
            else:
                for rep in range(reps):
                    emit_body(f"_r{rep}")

    nc.compile()
    return nc


def host_prep(qkv_w, lambda_param, gn_weight, gn_bias, proj_w, proj_b):
    qkv_w = np.asarray(qkv_w, np.float32)
    lam = np.asarray(lambda_param, np.float32).reshape(NH)
    gw = np.asarray(gn_weight, np.float32)
    gbv = np.asarray(gn_bias, np.float32)

    W_qk = np.zeros((C, 1024), np.float32)
    kinds = [("q1", 0), ("q1", 1), ("q2", 0), ("q2", 1),
             ("k1", 0), ("k1", 1), ("k2", 0), ("k2", 1)]
    d = np.arange(HD2)
    for t, (kind, hg) in enumerate(kinds):
        for j in range(4):
            h = 4 * hg + j
            if kind == "q1":
                rows, mul = h * HD + d, SCALE
            elif kind == "q2":
                rows, mul = h * HD + HD2 + d, 1.0
            elif kind == "k1":
                rows, mul = C + h * HD + d, 1.0
            else:
                rows, mul = C + h * HD + HD2 + d, 1.0
            W_qk[:, t * 128 + 32 * j + d] = (qkv_w[rows] * mul).T

    W_v = np.zeros((C, 512), np.float32)
    dd = np.arange(HD)
    for h in range(NH):
        W_v[:, 64 * h + dd] = qkv_w[2 * C + h * HD + dd].T

    W_projT = np.ascontiguousarray(np.asarray(proj_w, np.float32).T)
    pbias = np.asarray(proj_b, np.float32).reshape(C, 1)

    ga = np.zeros((2, 128, 1), np.float32)
    gb = np.zeros((2, 128, 1), np.float32)
    for g in range(2):
        for j in range(4):
            h = 4 * g + j
            ch = h * HD2 + d
            ga[g, 32 * j + d, 0] = gw[ch] * SCALE * lam[h]
            gb[g, 32 * j + d, 0] = gbv[ch] * SCALE * lam[h]
    return dict(wqk=W_qk, wv=W_v, wproj=W_projT, pbias=pbias, ga=ga, gb=gb)


_CACHE = {}


def kernel(x, qkv_w, lambda_param, gn_weight, gn_bias, proj_w, proj_b):
    B = x.shape[0]
    assert B == 8
    xf = np.asarray(x, np.float32).reshape(B, C, N)
    shared = host_prep(qkv_w, lambda_param, gn_weight, gn_bias, proj_w, proj_b)
    if "nc" not in _CACHE:
        _CACHE["nc"] = build_nc()
    nc = _CACHE["nc"]
    in_maps = [dict(x=xf[b], **shared) for b in range(B)]
    res = bass_utils.run_bass_kernel_spmd(nc, in_maps, list(range(8)))
    out = np.stack([res.results[b]["out"] for b in range(B)])
    return out.reshape(B, C, 32, 32).astype(np.float32)
